# revision 11
# baseline (speedup 1.0000x reference)
"""Multi-head attention (4x2048x512, 8 heads of 64) on 8 Trainium2 NeuronCores.

Sharding: core c handles batch b = c//2 and head-group g = c%2 (4 heads each).
The host pre-transposes x[b] -> xT [512, 2048] and slices the QKV / out
projection weights per core.  Each core computes

    qT,kT  = w_qk.T @ xT          (per head, [64, 2048] each)
    v      = xT.T  @ w_v          (natural [2048, 256], +ones column)
    sT     = kT.T  @ qT           (scores transposed, [j, i])
    p      = exp(sT / 8)          (flash-style over j-chunks)
    oT     = v_aug.T @ p          (accumulated over j; row 64 = softmax denom)
    attT   = oT[0:64] / denom
    yT     = w_out_slice.T @ attT  ([512, 2048] partial)

and the host reduces: out[b] = (yT[2b] + yT[2b+1]).T + b_out.

All matmuls run as float32r (full PE rate at free-dim >= 256, ~tf32
precision).  The softmax exp runs on the Scalar engine straight out of PSUM
with the 1/8 scale folded into the activation.
"""

import numpy as np

N = 2048          # sequence length
DMODEL = 512      # model dim
DH = 64           # head dim
HEADS = 4         # heads per core
N_CORES = 8
I_HALF = N // 2   # flash loop processes i in halves of 1024
JC = N // 128     # 16 j-chunks per head
KO = DMODEL // 128  # 4 contraction chunks of the model dim

_CACHE = {}


def _fixup_drains(nc, mybir):
    """walrus in this container rejects instructions carrying multiple sem
    waits ("Too many sync wait commands", e.g. on Drain and on the fused
    LDWEIGHTS of Matmult); hoist all-but-one wait onto single-wait NoOps
    right before the instruction — semantically identical (the engine
    stalls at the NoOps instead)."""
    for fn in nc.m.functions:
        for blk in fn.blocks:
            new = []
            for inst in blk.instructions:
                si = getattr(inst, "sync_info", None)
                if si is not None and si.on_wait:
                    keep = 0 if isinstance(inst, mybir.InstDrain) else 1
                    waits = list(si.on_wait)
                    if len(waits) > keep:
                        extra, rest = waits[keep:], waits[:keep]
                        for j, w in enumerate(extra):
                            nop = mybir.InstNoOp(
                                name=f"{inst.name}-ws{j}", ins=[], outs=[]
                            )
                            nop.engine = inst.engine
                            nop.sync_info = mybir.SyncInfo(on_wait=[w], on_update=[])
                            new.append(nop)
                        si.on_wait = rest
                new.append(inst)
            blk.instructions = new


def build_nc(repeat=1, fixup=True):
    """Build the per-core Bass program (identical on all 8 cores)."""
    import concourse.bass as bass
    import concourse.tile as tile
    from concourse import mybir

    f32 = mybir.dt.float32

    def r_(ap):  # reinterpret fp32 as float32r for full-rate PE matmuls
        return ap.bitcast(mybir.dt.float32r)

    nc = bass.Bass()
    xt = nc.dram_tensor("xt", [DMODEL, N], f32, kind="ExternalInput")
    wqk = nc.dram_tensor("wqk", [DMODEL, HEADS * 128], f32, kind="ExternalInput")
    wv = nc.dram_tensor("wv", [DMODEL, HEADS * DH], f32, kind="ExternalInput")
    wo = nc.dram_tensor("wo", [HEADS * DH, DMODEL], f32, kind="ExternalInput")
    yt = nc.dram_tensor("yt", [DMODEL, N], f32, kind="ExternalOutput")

    with tile.TileContext(nc) as tc:
        with (
            tc.tile_pool(name="singles", bufs=1) as singles,
            tc.tile_pool(name="dram", bufs=1, space="DRAM") as dram,
        ):
            x_sb = singles.tile([128, KO, N], f32)
            wqk_sb = singles.tile([128, KO, HEADS, 128], f32)
            wv_sb = singles.tile([128, KO, HEADS * DH], f32)
            wo_sb = singles.tile([128, 2, DMODEL], f32)
            q_sb = singles.tile([DH, HEADS, N], f32)
            k_sb = singles.tile([DH, HEADS, N], f32)
            v_sb = singles.tile([128, JC, HEADS, DH + 1], f32)
            att_sb = singles.tile([128, 2, N], f32)
            den_sp = singles.tile([128, 2, HEADS, I_HALF // 128], f32)
            r_sp = singles.tile([128, 2, HEADS, I_HALF // 128], f32)
            r_rep = singles.tile([128, 2, N], f32)
            dens_dram = dram.tile([2, HEADS, I_HALF], f32)
            r_dram = dram.tile([2, HEADS, I_HALF], f32)

            nc.sync.dma_start(r_(x_sb[:]), r_(xt.ap().rearrange("(ko p) n -> p ko n", p=128)))
            nc.sync.dma_start(
                r_(wqk_sb[:]), r_(wqk.ap().rearrange("(ko p) (h m) -> p ko h m", p=128, m=128))
            )
            nc.sync.dma_start(
                r_(wv_sb[:]), r_(wv.ap().rearrange("(ko p) v -> p ko v", p=128))
            )
            nc.sync.dma_start(
                r_(wo_sb[:]), r_(wo.ap().rearrange("(c p) n -> p c n", p=128))
            )
            ones_sb = singles.tile([128, JC, HEADS, 1], f32)
            nc.vector.memset(ones_sb[:], 1.0)
            nc.vector.tensor_copy(r_(v_sb[:, :, :, DH : DH + 1]), ones_sb[:])

            for rep in range(repeat):
                # ---- phase A: qkT per head + v (natural layout) ----
                with (
                    tc.tile_pool(name="ps_qk", bufs=1, space="PSUM") as ps_qk,
                    tc.tile_pool(name="ps_v", bufs=4, space="PSUM") as ps_v,
                ):
                    for h in range(HEADS):
                        pqk = ps_qk.tile([128, N], f32)
                        for t in range(N // 512):
                            for ko in range(KO):
                                nc.tensor.matmul(
                                    pqk[:, t * 512 : (t + 1) * 512],
                                    r_(wqk_sb[:, ko, h, :]),
                                    r_(x_sb[:, ko, t * 512 : (t + 1) * 512]),
                                    start=(ko == 0),
                                    stop=(ko == KO - 1),
                                )
                        nc.vector.tensor_copy(r_(q_sb[:, h, :]), pqk[0:DH, :])
                        nc.vector.tensor_copy(r_(k_sb[:, h, :]), pqk[DH:128, :])
                    for jc in range(JC):
                        pv = ps_v.tile([128, HEADS * DH], f32)
                        for ko in range(KO):
                            nc.tensor.matmul(
                                pv[:],
                                r_(x_sb[:, ko, jc * 128 : (jc + 1) * 128]),
                                r_(wv_sb[:, ko, :]),
                                start=(ko == 0),
                                stop=(ko == KO - 1),
                            )
                        for h in range(HEADS):
                            nc.vector.tensor_copy(
                                r_(v_sb[:, jc, h, 0:DH]), pv[:, h * DH : (h + 1) * DH]
                            )

                # ---- phase B: flash attention over (i_half, head, j-chunk) ----
                with (
                    tc.tile_pool(name="ps_s", bufs=2, space="PSUM") as ps_s,
                    tc.tile_pool(name="ps_o", bufs=2, space="PSUM") as ps_o,
                    tc.tile_pool(name="p_sb", bufs=3) as p_pool,
                    tc.tile_pool(name="den_row", bufs=2) as den_pool,
                ):
                    for ih in range(2):
                        i0 = ih * I_HALF
                        for h in range(HEADS):
                            o = ps_o.tile([DH + 1, I_HALF], f32)
                            for jc in range(JC):
                                s = ps_s.tile([128, I_HALF], f32)
                                for t in range(I_HALF // 512):
                                    nc.tensor.matmul(
                                        s[:, t * 512 : (t + 1) * 512],
                                        r_(k_sb[:, h, jc * 128 : (jc + 1) * 128]),
                                        r_(q_sb[:, h, i0 + t * 512 : i0 + (t + 1) * 512]),
                                        start=True,
                                        stop=True,
                                    )
                                p = p_pool.tile([128, I_HALF], f32)
                                nc.scalar.activation(
                                    r_(p[:]), s[:], mybir.ActivationFunctionType.Exp,
                                    scale=0.125,
                                )
                                for t in range(I_HALF // 512):
                                    nc.tensor.matmul(
                                        o[:, t * 512 : (t + 1) * 512],
                                        r_(v_sb[:, jc, h, :]),
                                        r_(p[:, t * 512 : (t + 1) * 512]),
                                        start=(jc == 0),
                                        stop=(jc == JC - 1),
                                    )
                            nc.vector.tensor_copy(
                                r_(att_sb[(h % 2) * DH : (h % 2 + 1) * DH, h // 2,
                                          i0 : i0 + I_HALF]),
                                o[0:DH, :],
                            )
                            den_row = den_pool.tile([1, I_HALF], f32)
                            nc.vector.tensor_copy(den_row[:], o[DH : DH + 1, :])
                            nc.sync.dma_start(
                                dens_dram[ih : ih + 1, h, :], den_row[0:1, :]
                            )

                # ---- normalize: recip of denominators, broadcast, multiply ----
                nc.sync.dma_start(
                    den_sp[:],
                    dens_dram[:].rearrange("ih h (p f) -> p ih h f", p=128),
                )
                nc.vector.reciprocal(r_sp[:], den_sp[:])
                nc.sync.dma_start(
                    r_dram[:].rearrange("ih h (p f) -> p ih h f", p=128),
                    r_sp[:],
                )
                for c in range(2):
                    for half in range(2):
                        h = 2 * c + half
                        nc.sync.dma_start(
                            r_rep[half * DH : (half + 1) * DH, c, :].rearrange(
                                "p (ih i) -> p ih i", ih=2
                            ),
                            bass.AP(
                                tensor=r_dram.tensor,
                                offset=r_dram.offset + h * I_HALF,
                                ap=[[0, DH], [HEADS * I_HALF, 2], [1, I_HALF]],
                            ),
                        )
                nc.vector.tensor_mul(r_(att_sb[:]), att_sb[:], r_rep[:])

                # ---- phase C: output projection yT = wo.T @ attT ----
                with (
                    tc.tile_pool(name="ps_y", bufs=4, space="PSUM") as ps_y,
                    tc.tile_pool(name="y_sb", bufs=2) as y_pool,
                ):
                    for m in range(KO):
                        y_row = y_pool.tile([128, N], f32)
                        for t in range(N // 512):
                            py = ps_y.tile([128, 512], f32)
                            for c in range(2):
                                nc.tensor.matmul(
                                    py[:],
                                    r_(wo_sb[:, c, m * 128 : (m + 1) * 128]),
                                    r_(att_sb[:, c, t * 512 : (t + 1) * 512]),
                                    start=(c == 0),
                                    stop=(c == 1),
                                )
                            nc.vector.tensor_copy(y_row[:, t * 512 : (t + 1) * 512], py[:])
                        nc.sync.dma_start(
                            yt.ap().rearrange("(mo p) n -> p mo n", p=128)[:, m, :],
                            y_row[:],
                        )

    if fixup:
        _fixup_drains(nc, mybir)
    return nc


def _per_core_inputs(x, w_qkv, w_out):
    """Slice + transpose the full inputs into the 8 per-core input maps."""
    ins = []
    for c in range(N_CORES):
        b, g = c // 2, c % 2
        xt = np.ascontiguousarray(x[b].T)                      # [512, 2048]
        wq = w_qkv[:, g * 256 : (g + 1) * 256]                 # [512, 256]
        wk = w_qkv[:, 512 + g * 256 : 512 + (g + 1) * 256]
        wv = w_qkv[:, 1024 + g * 256 : 1024 + (g + 1) * 256]
        # per head: [w_q_h | w_k_h] -> [512, 4, 128]
        wqk = np.empty((DMODEL, HEADS, 128), np.float32)
        for h in range(HEADS):
            wqk[:, h, :DH] = wq[:, h * DH : (h + 1) * DH]
            wqk[:, h, DH:] = wk[:, h * DH : (h + 1) * DH]
        wo = w_out[g * 256 : (g + 1) * 256, :]                 # [256, 512]
        ins.append(
            {
                "xt": xt,
                "wqk": np.ascontiguousarray(wqk.reshape(DMODEL, HEADS * 128)),
                "wv": np.ascontiguousarray(wv),
                "wo": np.ascontiguousarray(wo),
            }
        )
    return ins


def run_on_hw(x, w_qkv, w_out, b_out, repeat=1):
    from concourse.bass_utils import run_bass_kernel_spmd

    key = ("nc", repeat)
    if key not in _CACHE:
        _CACHE[key] = build_nc(repeat=repeat)
    nc = _CACHE[key]
    ins = _per_core_inputs(
        np.asarray(x, np.float32),
        np.asarray(w_qkv, np.float32),
        np.asarray(w_out, np.float32),
    )
    res = run_bass_kernel_spmd(nc, ins, core_ids=list(range(N_CORES)))
    yts = [res.results[c]["yt"] for c in range(N_CORES)]
    b_out = np.asarray(b_out, np.float32)
    out = np.stack(
        [(yts[2 * b] + yts[2 * b + 1]).T + b_out[None, :] for b in range(4)]
    )
    return out.astype(np.float32)


def kernel(x, w_qkv, w_out, b_out):
    return run_on_hw(x, w_qkv, w_out, b_out, repeat=1)
